# revision 21
# baseline (speedup 1.0000x reference)
"""GCN (4-layer, DGL GraphConv norm='both' + BatchNorm + residual + mean
readout + MLP) on 8 Trainium2 NeuronCores via Bass/Tile.

v3 strategy (dst-sharded, 12544 padded rows/core):
- bf16 gather table split into 4 chunk tensors (one per SWDGE queue), each
  refreshed by its own sub-AllGather issued as soon as the corresponding
  quarter of the local shard is ready - so next-layer gathers overlap the
  BN-apply phase and the collectives.
- Edge->dst one-hot (norm weights baked in) precomputed on the HOST in
  bf16, streamed from DRAM in 8KB-per-partition lines.
- Edge aggregation is PE matmul bf16xbf16->fp32 PSUM; gathers are emitted
  round-robin across the 4 queues to keep all Q7 core pairs grinding.
- x (feature-major fp32) lives in SBUF across all layers; embedding input
  arrives host-pre-transposed; the mean-readout membership matmul is
  folded into layer 3's apply loop.
"""

import os
import sys
import types
import numpy as np

# ---------------------------------------------------------------- problem dims
N = 100000
E_FULL = 1600000
G = 128
HID = 128
L = 4
NC = 8
EPS = 1e-5
V = N // NC                 # 12500 real nodes per core
WPC = (V + 127) // 128      # 98 windows per core
VP = WPC * 128              # 12544 padded nodes per core
NPAD = NC * VP              # 100352 padded global rows
NCHUNK = 4
GRP = 4                     # windows per dense group
OHB = 32                    # one-hot chunks per DMA batch
IDXG = 896                  # indices per dma_gather instruction
GPT = IDXG // 128           # chunks per gather tile
PREK = 3                    # gathers pre-issued right after each AllGather
CHUNK = NPAD // NCHUNK      # 25088 (< 2**15, int16-indexable)


# ------------------------------------------------------------- compile patches
def _apply_patches():
    """This walrus build accepts only one sync-wait per instruction; hoist
    extra waits emitted by the Tile scheduler onto single-wait NoOps."""
    import concourse.mybir as mb
    from concourse.tile import TileContext, ScopedClock

    if getattr(TileContext, "_gcn_patched", False):
        return
    orig = TileContext._commit_and_lower

    def _split_waits(self, inst, *args):
        si = getattr(inst, "sync_info", None)
        if si is not None:
            waits = list(si.on_wait or [])
            if len(waits) > 1:
                for w in waits[:-1]:
                    nop = mb.InstNoOp(
                        name=self.nc.get_next_instruction_name(), ins=[], outs=[]
                    )
                    nop.engine = inst.engine
                    nop.sync_info = mb.SyncInfo(on_wait=[w], on_update=[])
                    orig(self, nop, *args)
                inst.sync_info = mb.SyncInfo(
                    on_wait=[waits[-1]], on_update=list(si.on_update or [])
                )
        return orig(self, inst, *args)

    def _drain_and_barrier(self, tick_clock, wait_clock):
        nop = self.nc.sync.nop(nofuse=True)
        inst = nop.ins
        wait_clock.add_sem_waits(inst, ScopedClock({None: tick_clock.global_clock}))
        si = inst.sync_info
        waits = list(si.on_wait) if si is not None else []
        inst.sync_info = mb.SyncInfo(on_wait=waits[:1], on_update=[])
        for w in waits[1:]:
            n2 = self.nc.sync.nop(nofuse=True)
            n2.ins.sync_info = mb.SyncInfo(on_wait=[w], on_update=[])
        self.nc.sync.drain()
        self.nc.all_engine_barrier()
        assert self.sems is not None
        popped = self.nc._tile_sem_poison_stack.pop()
        assert popped is self._sem_poison
        self.nc.clear_and_free_semaphores(list(self.sems.allocated().values()))
        self.nc.all_engine_barrier()

    TileContext._commit_and_lower = _split_waits
    TileContext._drain_and_barrier = _drain_and_barrier
    TileContext._gcn_patched = True


# --------------------------------------------------------- host preprocessing
def build_plan(src, dst, graph_id):
    import ml_dtypes
    bf16 = ml_dtypes.bfloat16
    f8 = ml_dtypes.float8_e3m4

    src = np.asarray(src).astype(np.int64)
    dst = np.asarray(dst).astype(np.int64)
    graph_id = np.asarray(graph_id).astype(np.int64)

    deg_out = np.bincount(src, minlength=N).astype(np.float64)
    deg_in = np.bincount(dst, minlength=N).astype(np.float64)
    wvec = (
        1.0
        / np.sqrt(np.maximum(deg_out, 1.0)[src] * np.maximum(deg_in, 1.0)[dst])
    ).astype(np.float32)

    srcrow = (src // V) * VP + (src % V)      # padded global row
    chunk = srcrow // CHUNK
    idxloc = (srcrow - chunk * CHUNK).astype(np.int16)

    core = dst // V
    dloc = dst - core * V
    win = dloc // 128
    dcol = (dloc % 128).astype(np.int32)

    # bucket edges per (core, chunk, window); bucket size = max over cores
    counts = np.zeros((NC, NCHUNK, WPC), np.int64)
    np.add.at(counts, (core, chunk, win), 1)
    smax = np.max(counts, axis=0)                    # [NCHUNK, WPC] slots
    # every window needs at least one slot overall so PSUM gets cleared
    empty_w = smax.sum(axis=0) == 0
    smax[0, empty_w] = 1

    off = np.zeros((NCHUNK, WPC + 1), np.int64)      # slot offsets per stream
    off[:, 1:] = np.cumsum(smax, axis=1)
    Sc = off[:, -1]                           # real slots per stream
    Cc = (Sc + 127) // 128                    # chunks per stream
    Ec = Cc * 128                             # slots incl. chunk-tail pad
    EcP = ((Ec + IDXG - 1) // IDXG) * IDXG    # gather-padded stream length

    # matmul segments per (stream, window): (chunk j, one-hot block bcol).
    # Full-128-row matmuls; a chunk crossing a window boundary gets a
    # separate one-hot column block per window (out-of-window rows zero).
    segs = [[[] for _ in range(WPC)] for _ in range(NCHUNK)]
    blkrange = [[] for _ in range(NCHUNK)]   # per block: (slot a, slot b)
    for c in range(NCHUNK):
        for w in range(WPC):
            a, b = int(off[c, w]), int(off[c, w + 1])
            while a < b:
                j = a // 128
                r1 = min(b - j * 128, 128)
                bcol = len(blkrange[c])
                blkrange[c].append((a, j * 128 + r1))
                segs[c][w].append((j, bcol))
                a = j * 128 + r1

    # per-core packed arrays
    order = np.lexsort((idxloc, win, chunk, core))  # stable grouping
    s_src = idxloc[order]
    s_dcol = dcol[order]
    s_w = wvec[order]
    # boundaries per (core, chunk, window)
    starts = np.zeros((NC, NCHUNK, WPC), np.int64)
    flatc = counts.reshape(-1)
    starts.reshape(-1)[1:] = np.cumsum(flatc)[:-1]

    T16 = int(EcP.max()) // 16
    NBLK = np.array([len(blkrange[c]) for c in range(NCHUNK)], np.int64)
    Ctot = int(NBLK.sum())                    # one-hot column blocks
    cbase = np.concatenate([[0], np.cumsum(NBLK)[:-1]])
    idx16 = np.full((NC, 128, T16), -1, np.int16)
    onehot = np.empty((NC, 128, Ctot * 128), f8)

    for r in range(NC):
        oh32 = np.zeros((128, Ctot * 128), np.float32)
        for c in range(NCHUNK):
            ii = np.zeros(int(EcP[c]), np.int16)
            delta = np.full(int(Ec[c]), -1, np.int32)
            ww = np.zeros(int(Ec[c]), np.float32)
            for w in range(WPC):
                cnt = counts[r, c, w]
                s0 = starts[r, c, w]
                pos = int(off[c, w])
                ii[pos:pos + cnt] = s_src[s0:s0 + cnt]
                delta[pos:pos + cnt] = s_dcol[s0:s0 + cnt]
                ww[pos:pos + cnt] = s_w[s0:s0 + cnt]
            ii[int(Ec[c]):] = -1                       # gather tail skip
            a = ii.reshape(-1, 16).T                   # [16, EcP/16]
            idx16[r, 32 * c:32 * c + 32, : a.shape[1]] = np.tile(a, (2, 1))
            # scatter weights into per-block one-hot columns
            for bcol, (sa, sb) in enumerate(blkrange[c]):
                s = np.arange(sa, sb, dtype=np.int64)
                dl = delta[sa:sb]
                valid = dl >= 0
                col0 = (int(cbase[c]) + bcol) * 128
                oh32[(s % 128)[valid], col0 + dl[valid]] = ww[sa:sb][valid] * 8.0
        onehot[r] = oh32.astype(f8)

    # graph ids per core window layout [128, WPC], pad = -1
    gid_p = np.full((NC, 128, WPC), -1.0, np.float32)
    for r in range(NC):
        g = graph_id[r * V:(r + 1) * V].astype(np.float32)
        gp = np.full(VP, -1.0, np.float32)
        gp[:V] = g
        gid_p[r] = gp.reshape(WPC, 128).T

    cnts = np.maximum(np.bincount(graph_id, minlength=G).astype(np.float32), 1.0)
    recip = np.tile((1.0 / cnts)[None, :], (128, 1)).astype(np.float32)

    return dict(
        segs=segs, Cc=Cc, Ec=Ec, EcP=EcP, Ctot=Ctot, cbase=cbase,
        nblk=NBLK, off=off, idx16=idx16, onehot=onehot, gid=gid_p,
        recip=recip,
    )


# ------------------------------------------------------------ program builder
def build_program(plan):
    _apply_patches()
    import concourse.bacc as bacc
    import concourse.mybir as mybir
    from concourse.tile import TileContext

    f32 = mybir.dt.float32
    bf16 = mybir.dt.bfloat16
    f8 = mybir.dt.float8e3
    i16 = mybir.dt.int16
    AX = mybir.AxisListType.X
    OP = mybir.AluOpType
    AF = mybir.ActivationFunctionType

    segs = plan["segs"]
    off = plan["off"]
    nblk = plan["nblk"]
    Cc = plan["Cc"]
    Ec = plan["Ec"]
    EcP = plan["EcP"]
    Ctot = int(plan["Ctot"])
    cbase = plan["cbase"]
    T16 = int(EcP.max()) // 16
    NGRP = (WPC + GRP - 1) // GRP
    NG = [int(EcP[c]) // IDXG for c in range(4)]   # gathers per stream
    LOOKAHEAD = 8

    nc = bacc.Bacc("TRN2", target_bir_lowering=False, debug=False,
                   enable_asserts=False, num_devices=NC, num_swdge_queues=4)

    # ---- external inputs
    ext = {}

    def inp(name, shape, dt=f32):
        ext[name] = nc.dram_tensor(name, list(shape), dt, kind="ExternalInput")
        return ext[name]

    hT_d = inp("hT", [128, VP], bf16)
    idx_d = inp("idx16", [128, T16], i16)
    oh_d = inp("onehot", [128, Ctot * 128], f8)
    gid_d = inp("gid", [128, WPC])
    recip_d = inp("recip", [128, G])
    iota_d = inp("iota", [128, 128])
    ident_d = inp("ident", [128, 128])
    wemb_d = inp("W_embed", [HID, HID], bf16)
    bemb_d = inp("b_embed", [HID, 1])
    wl_d = [inp(f"Wl{i}", [HID, HID], bf16) for i in range(L)]
    gam_d = inp("gammas", [HID, L])
    bet_d = inp("betas", [HID, L])
    w1_d = inp("W1", [128, 64])
    b1_d = inp("b1", [64, 1])
    w2_d = inp("W2", [64, 32])
    b2_d = inp("b2", [32, 1])
    w3_d = inp("W3", [32, 10])
    b3_d = inp("b3", [10, 1])

    out_d = nc.dram_tensor("out", [10, G], f32, kind="ExternalOutput")

    # ---- internal DRAM
    x_all = [nc.dram_tensor(f"x_all{t}", [NPAD, HID], bf16, addr_space="Shared")
             for t in range(L)]
    xr_b = [nc.dram_tensor(f"xr{t}", [VP, HID], bf16) for t in range(L)]
    ar_in = [nc.dram_tensor(f"arin{i}", [128, 2], f32) for i in range(L)]
    ar_out = [nc.dram_tensor(f"arout{i}", [128, 2], f32, addr_space="Shared")
              for i in range(L)]
    hg_in = nc.dram_tensor("hgin", [128, G], f32)
    hg_out = nc.dram_tensor("hgout", [128, G], f32, addr_space="Shared")

    RG = [list(range(NC))]

    with TileContext(nc) as tc:
        cp = tc.alloc_tile_pool(name="const", bufs=1)
        wp = tc.alloc_tile_pool(name="work", bufs=3)
        mp = tc.alloc_tile_pool(name="moh", bufs=2)
        gp = [tc.alloc_tile_pool(name=f"gs{c}", bufs=5) for c in range(4)]
        pp = tc.alloc_tile_pool(name="ps", bufs=5, space="PSUM")
        pp2 = tc.alloc_tile_pool(name="ps2", bufs=2, space="PSUM")
        pph = tc.alloc_tile_pool(name="psh", bufs=1, space="PSUM")

        def load_const(name, dram, shape, dt=f32):
            t = cp.tile(list(shape), dt, tag=name)
            nc.sync.dma_start(out=t[:], in_=dram[:, :])
            return t

        idx_t = load_const("idx", idx_d, [128, T16], i16)
        gid_t = load_const("gid", gid_d, [128, WPC])
        recip_t = load_const("recip", recip_d, [128, G])
        iota_t = load_const("iota", iota_d, [128, 128])
        ident_t = load_const("ident", ident_d, [128, 128])
        wemb_t = load_const("wemb", wemb_d, [HID, HID], bf16)
        bemb_t = load_const("bemb", bemb_d, [HID, 1])
        wl_t = [load_const(f"wl{i}", wl_d[i], [HID, HID], bf16) for i in range(L)]
        gam_t = load_const("gam", gam_d, [HID, L])
        bet_t = load_const("bet", bet_d, [HID, L])
        w1_t = load_const("w1", w1_d, [128, 64])
        b1_t = load_const("b1", b1_d, [64, 1])
        w2_t = load_const("w2", w2_d, [64, 32])
        b2_t = load_const("b2", b2_d, [32, 1])
        w3_t = load_const("w3", w3_d, [32, 10])
        b3_t = load_const("b3", b3_d, [10, 1])

        eps_t = cp.tile([128, 1], f32, tag="eps")
        nc.vector.memset(eps_t[:], EPS)
        xT = cp.tile([128, VP], f32, tag="xT")       # resident features (FM)
        hnT = cp.tile([128, VP], f32, tag="hnT")
        ssum = cp.tile([128, NGRP], f32, tag="ssum")
        ssq = cp.tile([128, NGRP], f32, tag="ssq")

        # ---- per-layer gather/one-hot emission state
        gpos = [[0] * 4 for _ in range(L)]
        kpos = [[0] * 4 for _ in range(L)]
        gq = [[dict() for _ in range(4)] for _ in range(L)]
        mtile = [[None] * 4 for _ in range(L)]
        mbase = [[-1] * 4 for _ in range(L)]

        def emit_gather(l, c):
            g = gpos[l][c]
            t = gp[c].tile([128, GPT, 128], bf16, tag=f"gt{c}")
            cnt = min(IDXG, int(Ec[c]) - g * IDXG)
            nc.gpsimd.dma_gather(
                out_ap=t[:],
                in_ap=x_all[l][c * CHUNK:(c + 1) * CHUNK, :],
                idxs_ap=idx_t[:, g * (IDXG // 16):(g + 1) * (IDXG // 16)],
                num_idxs=IDXG, num_idxs_reg=cnt, elem_size=HID,
                queue_num=c)
            gq[l][c][g] = t
            gpos[l][c] += 1

        def ensure_onehot(l, c, bcol):
            blk = bcol // OHB
            if mbase[l][c] == blk:
                return
            col0 = (int(cbase[c]) + blk * OHB) * 128
            nb = min(OHB, int(nblk[c]) - blk * OHB)
            m = mp.tile([128, OHB * 128], f8, tag=f"m8_{c}")
            nc.sync.dma_start(out=m[:, :nb * 128],
                              in_=oh_d[:, col0:col0 + nb * 128])
            mtile[l][c] = m
            mbase[l][c] = blk

        def allgather_and_preissue(t):
            nc.gpsimd.collective_compute(
                "AllGather", mybir.AluOpType.bypass, replica_groups=RG,
                ins=[xr_b[t].ap().opt()], outs=[x_all[t].ap().opt()])
            for _ in range(PREK):
                for q in range(4):
                    if gpos[t][q] < NG[q]:
                        emit_gather(t, q)

        # ---------------------------------------------------------- embedding
        for gi in range(NGRP):
            gw = min(GRP, WPC - gi * GRP)
            wid = gw * 128
            sl = slice(gi * GRP * 128, gi * GRP * 128 + wid)
            ht = wp.tile([128, GRP * 128], bf16, tag="ht")
            nc.sync.dma_start(out=ht[:, :wid], in_=hT_d[:, sl])
            x0p = pp2.tile([128, GRP * 128], f32, tag="hnp")
            nc.tensor.matmul(out=x0p[:, :wid], lhsT=wemb_t[:], rhs=ht[:, :wid],
                             start=True, stop=True)
            nc.vector.tensor_scalar_add(out=xT[:, sl], in0=x0p[:, :wid],
                                        scalar1=bemb_t[:, 0:1])
            nmt4 = wp.tile([128, GRP, 128], bf16, tag="nmb")
            for wi in range(gw):
                w = gi * GRP + wi
                tpp = pp.tile([128, 128], f32, tag="mm128")
                nc.tensor.transpose(out=tpp[:],
                                    in_=xT[:, w * 128:(w + 1) * 128],
                                    identity=ident_t[:])
                nc.vector.tensor_copy(out=nmt4[:, wi, :], in_=tpp[:])
            r0 = gi * GRP * 128
            nc.sync.dma_start(
                out=xr_b[0][r0:r0 + gw * 128, :].rearrange(
                    "(j p) f -> p j f", p=128),
                in_=nmt4[:, :gw, :])
        allgather_and_preissue(0)

        # --------------------------------------------------------- GCN layers
        for l in range(L):
            for w in range(WPC):
                # pre-issue gathers round-robin across queues
                wl_ahead = min(w + LOOKAHEAD, WPC - 1)
                for c in range(4):
                    need_ch = (int(off[c, wl_ahead + 1]) + 127) // 128
                    need = (need_ch + GPT - 1) // GPT
                    if gpos[l][c] < need and gpos[l][c] < NG[c]:
                        emit_gather(l, c)
                psw = pp.tile([128, 128], f32, tag="mm128")
                tot = sum(len(segs[c][w]) for c in range(4))
                done = 0
                for c in range(4):
                    for (j, bcol) in segs[c][w]:
                        g = j // GPT
                        while g >= gpos[l][c]:
                            emit_gather(l, c)
                        ensure_onehot(l, c, bcol)
                        jt = j % GPT
                        jm = bcol % OHB
                        nc.tensor.matmul(
                            out=psw[:],
                            lhsT=gq[l][c][g][:, jt, :],
                            rhs=mtile[l][c][:, jm * 128:(jm + 1) * 128],
                            start=(done == 0), stop=(done == tot - 1))
                        done += 1
                gi, wi = w // GRP, w % GRP
                gw = min(GRP, WPC - gi * GRP)
                if wi == 0:
                    agg4 = wp.tile([128, GRP * 128], bf16, tag="agg4")
                nc.vector.tensor_scalar_mul(
                    out=agg4[:, wi * 128:(wi + 1) * 128], in0=psw[:],
                    scalar1=1.0 / 8.0)
                if wi == gw - 1:
                    wid = gw * 128
                    sl = slice(gi * GRP * 128, gi * GRP * 128 + wid)
                    hnp = pp2.tile([128, GRP * 128], f32, tag="hnp")
                    nc.tensor.matmul(out=hnp[:, :wid], lhsT=wl_t[l][:],
                                     rhs=agg4[:, :wid], start=True, stop=True)
                    nc.vector.tensor_copy(out=hnT[:, sl], in_=hnp[:, :wid])
                    sq = wp.tile([128, GRP * 128], f32, tag="sqt")
                    nc.vector.tensor_tensor(out=sq[:, :wid], in0=hnT[:, sl],
                                            in1=hnT[:, sl], op=OP.mult)
                    nc.vector.reduce_sum(out=ssum[:, gi:gi + 1],
                                         in_=hnT[:, sl], axis=AX)
                    nc.vector.reduce_sum(out=ssq[:, gi:gi + 1],
                                         in_=sq[:, :wid], axis=AX)

            # ----- BN stats + AllReduce
            stat_t = wp.tile([128, 2], f32, tag="stat")
            nc.vector.reduce_sum(out=stat_t[:, 0:1], in_=ssum[:], axis=AX)
            nc.vector.reduce_sum(out=stat_t[:, 1:2], in_=ssq[:], axis=AX)
            nc.sync.dma_start(out=ar_in[l][:, :], in_=stat_t[:])
            nc.gpsimd.collective_compute(
                "AllReduce", mybir.AluOpType.add, replica_groups=RG,
                ins=[ar_in[l].ap().opt()], outs=[ar_out[l].ap().opt()])
            st2 = wp.tile([128, 2], f32, tag="st2")
            nc.sync.dma_start(out=st2[:], in_=ar_out[l][:, :])
            mu = wp.tile([128, 1], f32, tag="mu")
            nc.vector.tensor_scalar_mul(out=mu[:], in0=st2[:, 0:1],
                                        scalar1=1.0 / N)
            var = wp.tile([128, 1], f32, tag="var")
            nc.vector.tensor_scalar_mul(out=var[:], in0=st2[:, 1:2],
                                        scalar1=1.0 / N)
            musq = wp.tile([128, 1], f32, tag="musq")
            nc.vector.tensor_tensor(out=musq[:], in0=mu[:], in1=mu[:],
                                    op=OP.mult)
            nc.vector.tensor_tensor(out=var[:], in0=var[:], in1=musq[:],
                                    op=OP.subtract)
            sd = wp.tile([128, 1], f32, tag="sd")
            nc.scalar.activation(out=sd[:], in_=var[:], func=AF.Sqrt,
                                 bias=eps_t[:, 0:1], scale=1.0)
            rstd = wp.tile([128, 1], f32, tag="rstd")
            nc.vector.reciprocal(out=rstd[:], in_=sd[:])
            scal = wp.tile([128, 1], f32, tag="scal")
            nc.vector.tensor_tensor(out=scal[:], in0=rstd[:],
                                    in1=gam_t[:, l:l + 1], op=OP.mult)
            shif = wp.tile([128, 1], f32, tag="shif")
            nc.vector.tensor_tensor(out=shif[:], in0=mu[:], in1=scal[:],
                                    op=OP.mult)
            nc.vector.tensor_tensor(out=shif[:], in0=bet_t[:, l:l + 1],
                                    in1=shif[:], op=OP.subtract)

            # ----- BN apply + relu + residual (in SBUF) + transpose out
            if l == L - 1:
                hgp = pph.tile([128, G], f32, tag="hgp")
            for gi in range(NGRP):
                gw = min(GRP, WPC - gi * GRP)
                wid = gw * 128
                sl = slice(gi * GRP * 128, gi * GRP * 128 + wid)
                act = wp.tile([128, GRP * 128], f32, tag="act")
                nc.scalar.activation(out=act[:, :wid], in_=hnT[:, sl],
                                     func=AF.Relu, scale=scal[:, 0:1],
                                     bias=shif[:, 0:1])
                nc.vector.tensor_tensor(out=xT[:, sl], in0=xT[:, sl],
                                        in1=act[:, :wid], op=OP.add)
                nmt4 = wp.tile([128, GRP, 128], bf16, tag="nmb")
                for wi in range(gw):
                    w = gi * GRP + wi
                    tpp = pp.tile([128, 128], f32, tag="mm128")
                    nc.tensor.transpose(out=tpp[:],
                                        in_=xT[:, w * 128:(w + 1) * 128],
                                        identity=ident_t[:])
                    nc.scalar.activation(out=nmt4[:, wi, :], in_=tpp[:],
                                         func=AF.Copy, scale=1.0, bias=0.0)
                    if l == L - 1:
                        gm = wp.tile([128, G], bf16, tag="gm")
                        nc.vector.tensor_scalar(
                            out=gm[:], in0=iota_t[:],
                            scalar1=gid_t[:, w:w + 1], scalar2=None,
                            op0=OP.is_equal)
                        nc.tensor.matmul(out=hgp[:], lhsT=nmt4[:, wi, :],
                                         rhs=gm[:],
                                         start=(w == 0), stop=(w == WPC - 1),
                                         skip_group_check=True)
                if l < L - 1:
                    r0 = gi * GRP * 128
                    nc.sync.dma_start(
                        out=xr_b[l + 1][r0:r0 + gw * 128, :].rearrange(
                            "(j p) f -> p j f", p=128),
                        in_=nmt4[:, :gw, :])
            if l < L - 1:
                allgather_and_preissue(l + 1)

        # ------------------------------------------------------------ readout
        hgs = wp.tile([128, G], f32, tag="hgs")
        nc.vector.tensor_copy(out=hgs[:], in_=hgp[:])
        nc.sync.dma_start(out=hg_in[:, :], in_=hgs[:])
        nc.gpsimd.collective_compute(
            "AllReduce", mybir.AluOpType.add, replica_groups=RG,
            ins=[hg_in.ap().opt()], outs=[hg_out.ap().opt()])
        hga = wp.tile([128, G], f32, tag="hga")
        nc.sync.dma_start(out=hga[:], in_=hg_out[:, :])
        nc.vector.tensor_tensor(out=hga[:], in0=hga[:], in1=recip_t[:],
                                op=OP.mult)
        t1p = pp2.tile([64, G], f32, tag="hnp")
        nc.tensor.matmul(out=t1p[:], lhsT=w1_t[:], rhs=hga[:],
                         start=True, stop=True)
        t1 = wp.tile([64, G], f32, tag="t1")
        nc.scalar.activation(out=t1[:], in_=t1p[:], func=AF.Relu,
                             bias=b1_t[:, 0:1], scale=1.0)
        t2p = pp2.tile([32, G], f32, tag="hnp")
        nc.tensor.matmul(out=t2p[:], lhsT=w2_t[:], rhs=t1[:],
                         start=True, stop=True)
        t2 = wp.tile([32, G], f32, tag="t2")
        nc.scalar.activation(out=t2[:], in_=t2p[:], func=AF.Relu,
                             bias=b2_t[:, 0:1], scale=1.0)
        t3p = pp2.tile([16, G], f32, tag="hnp")
        nc.tensor.matmul(out=t3p[:10, :], lhsT=w3_t[:], rhs=t2[:],
                         start=True, stop=True)
        ot = wp.tile([16, G], f32, tag="ot")
        nc.vector.tensor_scalar_add(out=ot[:10, :], in0=t3p[:10, :],
                                    scalar1=b3_t[:, 0:1])
        nc.sync.dma_start(out=out_d[:, :], in_=ot[:10, :])

        for p in [pph, pp2, pp, *reversed(gp), mp, wp, cp]:
            p.release()

    nc.compile()
    return nc


# ------------------------------------------------------------------- kernel()
def kernel(**inputs):
    _apply_patches()
    import jax
    jax.devices()
    _install_ntff_noop()
    import ml_dtypes
    from concourse.bass_utils import run_bass_kernel_spmd

    bf16 = ml_dtypes.bfloat16
    h = np.asarray(inputs["h"], np.float32)
    src = np.asarray(inputs["src"])
    dst = np.asarray(inputs["dst"])
    graph_id = np.asarray(inputs["graph_id"])

    plan = build_plan(src, dst, graph_id)
    nc = build_program(plan)

    iota = np.tile(np.arange(128, dtype=np.float32)[None, :], (128, 1))
    ident = np.eye(128, dtype=np.float32)

    Wl = np.asarray(inputs["Wl"], np.float32)
    in_maps = []
    for r in range(NC):
        hp = np.zeros((VP, HID), np.float32)
        hp[:V] = h[r * V:(r + 1) * V]
        m = {
            "hT": np.ascontiguousarray(hp.T).astype(bf16),
            "idx16": plan["idx16"][r],
            "onehot": plan["onehot"][r],
            "gid": plan["gid"][r],
            "recip": plan["recip"],
            "iota": iota,
            "ident": ident,
            "W_embed": np.asarray(inputs["W_embed"], np.float32).astype(bf16),
            "b_embed": np.asarray(inputs["b_embed"], np.float32).reshape(HID, 1),
            "gammas": np.asarray(inputs["gamma"], np.float32).T.copy(),
            "betas": np.asarray(inputs["beta"], np.float32).T.copy(),
            "W1": np.asarray(inputs["W1"], np.float32),
            "b1": np.asarray(inputs["b1"], np.float32).reshape(-1, 1),
            "W2": np.asarray(inputs["W2"], np.float32),
            "b2": np.asarray(inputs["b2"], np.float32).reshape(-1, 1),
            "W3": np.asarray(inputs["W3"], np.float32),
            "b3": np.asarray(inputs["b3"], np.float32).reshape(-1, 1),
        }
        for i in range(L):
            m[f"Wl{i}"] = Wl[i].astype(bf16)
        in_maps.append(m)

    trace = os.environ.get("GCN_TRACE") == "1"
    res = run_bass_kernel_spmd(nc, in_maps, core_ids=list(range(NC)),
                               trace=trace)
    if trace and res.exec_time_ns:
        print(f"HW exec time: {res.exec_time_ns} ns")
        if res.instructions_and_trace:
            print("trace:", res.instructions_and_trace[1])
    return np.ascontiguousarray(res.results[0]["out"].T)


def _install_ntff_noop():
    """bass_utils imports antenv.axon_hooks unconditionally when trace=True;
    provide the module (and, for GCN_TRACE=1, the real ctypes hook)."""
    if "antenv.axon_hooks" in sys.modules:
        return
    mod = types.ModuleType("antenv.axon_hooks")
    _h = [None]
    mod.set_axon_ntff_profile_hook = lambda h: _h.__setitem__(0, h)
    mod.get_axon_ntff_profile_hook = lambda: _h[0]
    sys.modules["antenv.axon_hooks"] = mod
    try:
        import antenv
        antenv.axon_hooks = mod
    except ImportError:
        pass
    if os.environ.get("GCN_TRACE") == "1":
        try:
            import ctypes
            from contextlib import contextmanager

            lib = ctypes.CDLL("/opt/axon/libaxon_pjrt.so")
            lib.axon_start_nrt_profile.argtypes = [
                ctypes.POINTER(ctypes.c_int64), ctypes.c_size_t]
            lib.axon_start_nrt_profile.restype = ctypes.c_int64
            lib.axon_stop_nrt_profile.argtypes = [ctypes.c_char_p]
            lib.axon_stop_nrt_profile.restype = ctypes.c_int64

            @contextmanager
            def ntff_profile(output_dir, device_ids=None):
                if device_ids:
                    ids = (ctypes.c_int64 * len(device_ids))(*device_ids)
                    rc = lib.axon_start_nrt_profile(ids, len(device_ids))
                else:
                    rc = lib.axon_start_nrt_profile(None, 0)
                if rc < 0:
                    raise RuntimeError(f"axon_start_nrt_profile rc={rc}")
                try:
                    yield
                finally:
                    n = lib.axon_stop_nrt_profile(str(output_dir).encode())
                    if n < 0:
                        raise RuntimeError(f"axon_stop_nrt_profile rc={n}")

            mod.set_axon_ntff_profile_hook(ntff_profile)
        except Exception:
            pass
